# revision 2
# baseline (speedup 1.0000x reference)
"""DeeperGCN forward on 8 Trainium2 NeuronCores (Bass/Tile) — v2.

Strategy (dst-node sharding, batched SWDGE gathers):
- 6250 nodes/core in 49 fixed blocks of 128 (last 106). Per layer each core
  computes its nodes' [P2|P1] = [msg*exp(t*msg) | exp(t*msg)] rows (f16,
  256B), split into group A (blocks 0..24) and B (25..48); AllGather builds
  two replicated tables tabA [25600,128] / tabB [24576,128] — each small
  enough for int16 dma_gather row indices.
- Edge phase: edges live at their dst core, grouped by (src-group, chunk of
  4 dst blocks). One dma_gather per (group, chunk) fetches ~36 tiles of 128
  src rows in a single SWDGE instruction (amortizes the ~1us fixed
  descriptor-gen cost that dominated v1). Aggregation per dst block is
  one-hot matmuls accumulated in PSUM: agg = num/max(den,1e-16) reproduces
  the softmax aggregation exactly.
- Node phase runs layer-wide per group with wide (multi-block) DVE/ACT ops:
  LN via segmented reduces over 3D views, single Exp/Sqrt activations per
  group, per-block PE transposes + f16 matmuls for the MLP.
"""

import json
import os
import sys
import types

import numpy as np

sys.path.insert(0, "/opt/trn_rl_repo")

# ---------------------------------------------------------------------------
# Workaround: this walrus build supports only ONE semaphore wait per
# instruction; Tile attaches several. Split extras onto NoOp instructions
# at BIR-JSON serialization time.
# ---------------------------------------------------------------------------
_PATCHED = False


def _install_bir_patch():
    global _PATCHED
    if _PATCHED:
        return
    _PATCHED = True
    import concourse.bass as bass

    orig = bass.Bass.to_json_bytes

    def patched(self):
        data = json.loads(orig(self).decode())
        ctr = 0
        for fn in data.get("functions", []):
            for bb in fn.get("blocks", []):
                new_insts = []
                for inst in bb.get("instructions", []):
                    si = inst.get("sync_info")
                    waits = (si or {}).get("on_wait") or []
                    if len(waits) > 1:
                        for w in waits[:-1]:
                            ctr += 1
                            nop = {
                                "engine": inst["engine"],
                                "ins": [],
                                "outs": [],
                                "name": f"{inst['name']}-sw{ctr}",
                                "opcode": "NoOp",
                                "sync_info": {"on_update": [], "on_wait": [w]},
                            }
                            if "debug" in inst:
                                nop["debug"] = inst["debug"]
                            new_insts.append(nop)
                        si["on_wait"] = [waits[-1]]
                    new_insts.append(inst)
                bb["instructions"] = new_insts
        return json.dumps(data).encode()

    bass.Bass.to_json_bytes = patched


def _install_trace_hook():
    """Optional: register the NTFF profiling hook (for test.py timing)."""
    import antenv

    if "antenv.axon_hooks" in sys.modules:
        return
    _m = types.ModuleType("antenv.axon_hooks")
    _m._hook = None
    _m.set_axon_ntff_profile_hook = lambda h: setattr(_m, "_hook", h)
    _m.get_axon_ntff_profile_hook = lambda: _m._hook
    sys.modules["antenv.axon_hooks"] = _m
    antenv.axon_hooks = _m
    try:
        from trn_agent_boot.trn_boot import _ntff_profile_via_ctypes

        _m._hook = _ntff_profile_via_ctypes("/opt/axon/libaxon_pjrt.so")
    except Exception:
        pass


N, NC, NPC = 50000, 8, 6250
H = 64
H2 = 128
F_IN = 128
LN_EPS = 1e-5
BN_EPS = 1e-5
GEN_EPS = 1e-7

NB = 49          # dst blocks per core (128 nodes, last has 106)
NA_BLK = 25      # group A: blocks 0..24
RA = NA_BLK * 128       # 3200 rows/core in table A
RB = (NB - NA_BLK) * 128  # 3072 rows/core in table B
TABA = NC * RA   # 25600
TABB = NC * RB   # 24576
CHUNK = 3        # dst blocks per gather chunk
NCOLS = NB * 128  # 6272

LAST_EXEC_NS = None


def _preprocess_edges(edge_index):
    """Cross-core-uniform tile geometry + per-core gather index / dst-label
    arrays.

    Returns (nt, gt0, chunks, TOT, MAXNT, idx16, drl) where
      nt[g][b]   tiles for (group g, dst block b)        (uniform)
      gt0[g][b]  global tile index of first tile of (g,b)
      chunks[g]  list of (b_lo, b_hi, ct0, ctiles) per gather chunk
      idx16      [NC, 128, TOT*8] int16 gather indices (16-wrapped, replicated)
      drl        [NC, 128, TOT] f16 dst position labels (255 = pad)
    """
    src = np.asarray(edge_index[0], dtype=np.int64)
    dst = np.asarray(edge_index[1], dtype=np.int64)
    c = dst // NPC
    ld = dst - c * NPC
    db = ld >> 7
    dp = ld & 127
    cs = src // NPC
    ls = src - cs * NPC
    bs = ls >> 7
    ps = ls & 127
    grp = (bs >= NA_BLK).astype(np.int64)
    row = np.where(
        grp == 0,
        cs * RA + ps * NA_BLK + bs,
        cs * RB + ps * (NB - NA_BLK) + (bs - NA_BLK),
    ).astype(np.int64)

    # per (core, grp, block) edge counts -> uniform tile counts
    key = (c * 2 + grp) * NB + db
    cnt = np.bincount(key, minlength=NC * 2 * NB).reshape(NC, 2, NB)
    nt = np.maximum(1, -(-cnt.max(axis=0) // 128))  # [2, NB] cross-core max

    # chunk partition of blocks (same for both groups)
    blos = list(range(0, NB, CHUNK))
    chunk_ranges = [(b0, min(b0 + CHUNK, NB)) for b0 in blos]

    # global tile numbering: group 0 chunks then group 1 chunks
    gt0 = np.zeros((2, NB), dtype=np.int64)
    chunks = [[], []]
    t = 0
    for g in range(2):
        for (b0, b1) in chunk_ranges:
            ct0 = t
            for b in range(b0, b1):
                gt0[g, b] = t
                t += int(nt[g, b])
            chunks[g].append((b0, b1, ct0, t - ct0))
    TOT = t
    MAXNT = int(nt.max())

    # per-edge flat slot: gt0[g,db]*128 + rank within (c,g,db)
    order = np.lexsort((db, grp, c))
    inv = np.empty_like(order)
    inv[order] = np.arange(len(order))
    # rank within each (c,g,db) bucket
    sorted_key = key[order]
    starts = np.searchsorted(sorted_key, np.arange(NC * 2 * NB), side="left")
    rank_sorted = np.arange(len(order)) - starts[sorted_key]
    rank = np.empty_like(rank_sorted)
    rank[order] = rank_sorted

    slot = gt0[grp, db] * 128 + rank  # within-core flat element slot

    idx16 = np.zeros((NC, 16, TOT * 8), dtype=np.int16)
    drl = np.full((NC, 128, TOT), 255.0, dtype=np.float16)
    for cc in range(NC):
        m = c == cc
        fl_idx = np.zeros(TOT * 128, dtype=np.int16)
        fl_drl = np.full(TOT * 128, 255, dtype=np.int64)
        fl_idx[slot[m]] = row[m].astype(np.int16)
        fl_drl[slot[m]] = dp[m]
        # idx wrap: element i -> [i%16, i//16]
        idx16[cc] = fl_idx.reshape(TOT * 8, 16).T
        # drl: element i of tile t -> [i%128, t]
        drl[cc] = fl_drl.reshape(TOT, 128).T.astype(np.float16)
    idx16 = np.tile(idx16, (1, 8, 1))  # replicate to 128 partitions
    return nt, gt0, chunks, TOT, MAXNT, idx16, drl


def kernel(
    x,
    edge_index,
    enc_W,
    enc_b,
    conv_t,
    conv_W1,
    conv_b1,
    conv_lng,
    conv_lnb,
    conv_W2,
    conv_b2,
    block_lng,
    block_lnb,
    fin_t,
    fin_W1,
    fin_b1,
    fin_bng,
    fin_bnb,
    fin_W2,
    fin_b2,
    _trace=False,
):
    global LAST_EXEC_NS
    _install_bir_patch()
    if _trace:
        _install_trace_hook()

    import concourse.bass as bass
    import concourse.mybir as mybir
    import concourse.tile as tile
    from concourse import library_config
    from concourse.bass_utils import run_bass_kernel_spmd
    from concourse.library_overlay import lower_extended_insts
    f32 = mybir.dt.float32
    f16 = mybir.dt.float16
    i16 = mybir.dt.int16
    AF = mybir.ActivationFunctionType
    OP = mybir.AluOpType
    AX = mybir.AxisListType

    x = np.asarray(x, dtype=np.float32)
    nt, gt0, chunks, TOT, MAXNT, idx16, drl = _preprocess_edges(
        np.asarray(edge_index)
    )
    NCH = len(chunks[0])  # chunks per group

    # ---------------- host-side parameter prep (replicated) ----------------
    rep = lambda v, w: np.ascontiguousarray(
        np.broadcast_to(np.asarray(v, np.float32).reshape(1, w), (128, w))
    )
    w1all = np.concatenate(
        [np.asarray(conv_W1, np.float32), np.asarray(fin_W1, np.float32)[None]], 0
    ).astype(np.float16)  # [5, 64, 128]
    w2all = np.concatenate(
        [np.asarray(conv_W2, np.float32), np.asarray(fin_W2, np.float32)[None]], 0
    ).astype(np.float16)  # [5, 128, 64]
    b1 = np.asarray(conv_b1, np.float32)
    b1c_list = [b1[i] - b1[i].mean() for i in range(4)] + [np.zeros(H2, np.float32)]
    b1c = np.concatenate([rep(v, H2) for v in b1c_list], axis=1)  # [128, 5*128]
    g_fin = np.asarray(fin_bng, np.float32) / np.sqrt(np.float32(1.0 + BN_EPS))
    garr = np.concatenate(
        [rep(v, H2) for v in list(np.asarray(conv_lng, np.float32)) + [g_fin]], axis=1
    )
    bar_fin = np.asarray(fin_b1, np.float32) * g_fin + np.asarray(fin_bnb, np.float32)
    barr = np.concatenate(
        [rep(v, H2) for v in list(np.asarray(conv_lnb, np.float32)) + [bar_fin]],
        axis=1,
    )
    b2r = np.concatenate(
        [rep(v, H) for v in list(np.asarray(conv_b2, np.float32)) + [np.asarray(fin_b2)]],
        axis=1,
    )  # [128, 5*64]
    blg = np.asarray(block_lng, np.float32)
    blb = np.asarray(block_lnb, np.float32)
    blkg = np.concatenate([rep(blg[i], H) for i in (1, 2, 3, 0)], axis=1)
    blkb = np.concatenate([rep(blb[i], H) for i in (1, 2, 3, 0)], axis=1)
    tvals = np.array(
        list(np.asarray(conv_t, np.float32)) + [float(np.asarray(fin_t))], np.float32
    )
    tsc = np.ascontiguousarray(np.broadcast_to(tvals.reshape(1, 5), (128, 5)))
    tbi = np.ascontiguousarray(tsc * np.float32(GEN_EPS))
    iota_rep = np.tile(
        np.arange(128, dtype=np.float16), (128, MAXNT)
    ).reshape(128, MAXNT * 128)
    encW = np.asarray(enc_W, np.float32).astype(np.float16)  # [128, 64]
    encb = rep(enc_b, H)

    # per-core transposed x: xT[c][f, b*128+p] = x[c*NPC + b*128 + p, f]
    xT = np.zeros((NC, 128, NCOLS), dtype=np.float16)
    for cc in range(NC):
        xc = x[cc * NPC : (cc + 1) * NPC]  # [6250, 128]
        xT[cc, :, : xc.shape[0]] = 0
        full = np.zeros((NCOLS, 128), np.float32)
        full[: xc.shape[0]] = xc
        xT[cc] = full.T.astype(np.float16)

    # ---------------- build the Bass program ----------------
    nc = bass.Bass(num_swdge_queues=4, dynamic_dma_scratch_size=32768)

    d_xT = nc.dram_tensor("xT", [128, NCOLS], f16, kind="ExternalInput")
    d_idx = nc.dram_tensor("idx16", [128, TOT * 8], i16, kind="ExternalInput")
    d_drl = nc.dram_tensor("drl", [128, TOT], f16, kind="ExternalInput")
    d_w1 = nc.dram_tensor("w1all", [5, H, H2], f16, kind="ExternalInput")
    d_w2 = nc.dram_tensor("w2all", [5, H2, H], f16, kind="ExternalInput")
    d_b1c = nc.dram_tensor("b1c", [128, 5 * H2], f32, kind="ExternalInput")
    d_gar = nc.dram_tensor("garr", [128, 5 * H2], f32, kind="ExternalInput")
    d_bar = nc.dram_tensor("barr", [128, 5 * H2], f32, kind="ExternalInput")
    d_b2r = nc.dram_tensor("b2r", [128, 5 * H], f32, kind="ExternalInput")
    d_blkg = nc.dram_tensor("blkg", [128, 4 * H], f32, kind="ExternalInput")
    d_blkb = nc.dram_tensor("blkb", [128, 4 * H], f32, kind="ExternalInput")
    d_tsc = nc.dram_tensor("tsc", [128, 5], f32, kind="ExternalInput")
    d_tbi = nc.dram_tensor("tbi", [128, 5], f32, kind="ExternalInput")
    d_lneps = nc.dram_tensor("lneps", [128, 1], f32, kind="ExternalInput")
    d_iota = nc.dram_tensor("iota", [128, MAXNT * 128], f16, kind="ExternalInput")
    d_ident = nc.dram_tensor("ident", [128, 128], f16, kind="ExternalInput")
    d_encW = nc.dram_tensor("encW", [128, H], f16, kind="ExternalInput")
    d_encb = nc.dram_tensor("encb", [128, H], f32, kind="ExternalInput")
    d_out = nc.dram_tensor("out", [NCOLS, H], f32, kind="ExternalOutput")

    d_TinA = nc.dram_tensor("T_in_a", [128, RA], f16)
    d_TinB = nc.dram_tensor("T_in_b", [128, RB], f16)
    d_tabA = [
        nc.dram_tensor(f"T_tabA{i}", [TABA, H2], f16, addr_space="Shared")
        for i in range(2)
    ]
    d_tabB = [
        nc.dram_tensor(f"T_tabB{i}", [TABB, H2], f16, addr_space="Shared")
        for i in range(2)
    ]
    d_tabs = [d_tabA, d_tabB]

    NBH = NB * H  # 3136

    # max tiles in one gather chunk
    TPG = max(int(ct) for g in range(2) for (_, _, _, ct) in chunks[g])

    with tile.TileContext(nc) as tc:
        nc.gpsimd.load_library(library_config.mlp)
        nidx_reg = nc.gpsimd.alloc_register(name="nidx")
        with (
            tc.tile_pool(name="state", bufs=1) as st,
            tc.tile_pool(name="wkw", bufs=1) as wk,
            tc.tile_pool(name="wks", bufs=2) as ws,
            tc.tile_pool(name="wide", bufs=1) as wd,
            tc.tile_pool(name="ga", bufs=2) as gpa,
            tc.tile_pool(name="gb", bufs=2) as gpb,
            tc.tile_pool(name="ohp", bufs=2) as ohp,
            tc.tile_pool(name="psa", bufs=3, space="PSUM") as pp,
            tc.tile_pool(name="psq", bufs=1, space="PSUM") as pq,
        ):
            # ---------------- persistent state / constants ----------------
            idx_sb = st.tile([128, TOT * 8], i16, tag="idx")
            nc.sync.dma_start(out=idx_sb[:], in_=d_idx[:])
            drl_sb = st.tile([128, TOT], f16, tag="drl")
            nc.sync.dma_start(out=drl_sb[:], in_=d_drl[:])
            iota_sb = st.tile([128, MAXNT * 128], f16, tag="iota")
            nc.sync.dma_start(out=iota_sb[:], in_=d_iota[:])
            ident = st.tile([128, 128], f16, tag="ident")
            nc.sync.dma_start(out=ident[:], in_=d_ident[:])
            b1c_sb = st.tile([128, 5 * H2], f32, tag="b1c")
            nc.sync.dma_start(out=b1c_sb[:], in_=d_b1c[:])
            gar_sb = st.tile([128, 5 * H2], f32, tag="gar")
            nc.sync.dma_start(out=gar_sb[:], in_=d_gar[:])
            bar_sb = st.tile([128, 5 * H2], f32, tag="bar")
            nc.sync.dma_start(out=bar_sb[:], in_=d_bar[:])
            b2r_sb = st.tile([128, 5 * H], f32, tag="b2r")
            nc.sync.dma_start(out=b2r_sb[:], in_=d_b2r[:])
            blkg_sb = st.tile([128, 4 * H], f32, tag="blkg")
            nc.sync.dma_start(out=blkg_sb[:], in_=d_blkg[:])
            blkb_sb = st.tile([128, 4 * H], f32, tag="blkb")
            nc.sync.dma_start(out=blkb_sb[:], in_=d_blkb[:])
            tsc_sb = st.tile([128, 5], f32, tag="tsc")
            nc.sync.dma_start(out=tsc_sb[:], in_=d_tsc[:])
            tbi_sb = st.tile([128, 5], f32, tag="tbi")
            nc.sync.dma_start(out=tbi_sb[:], in_=d_tbi[:])
            lneps_sb = st.tile([128, 1], f32, tag="lneps")
            nc.sync.dma_start(out=lneps_sb[:], in_=d_lneps[:])
            encb_sb = st.tile([128, H], f32, tag="encb")
            nc.sync.dma_start(out=encb_sb[:], in_=d_encb[:])
            encW_sb = st.tile([128, H], f16, tag="encW")
            nc.sync.dma_start(out=encW_sb[:], in_=d_encW[:])
            xT_sb = st.tile([128, NCOLS], f16, tag="xT")
            nc.sync.dma_start(out=xT_sb[:], in_=d_xT[:])
            tlocA = st.tile([128, RA], f16, tag="tlocA")
            tlocB = st.tile([128, RB], f16, tag="tlocB")

            h_a = st.tile([128, NBH], f32, tag="h_a")
            h_b = st.tile([128, NBH], f32, tag="h_b")
            r_sb = st.tile([128, NBH], f32, tag="r_sb")

            # ---------------- encoder: r = x @ enc_W + enc_b ----------------
            for b in range(NB):
                ps_e = pq.tile([128, H], f32, tag="pe")
                nc.tensor.matmul(
                    out=ps_e[:],
                    lhsT=xT_sb[:, b * 128 : (b + 1) * 128],
                    rhs=encW_sb[:],
                    start=True,
                    stop=True,
                )
                nc.scalar.copy(out=r_sb[:, b * H : (b + 1) * H], in_=ps_e[:])
            nc.vector.tensor_tensor(
                out=r_sb[:].rearrange("p (b f) -> p b f", f=H),
                in0=r_sb[:].rearrange("p (b f) -> p b f", f=H),
                in1=encb_sb[:].unsqueeze(1).to_broadcast([128, NB, H]),
                op=OP.add,
            )

            h_cur, h_nxt = h_a, h_b

            def t_chunk_group(g, lidx, initial):
                """Compute [P2|P1] rows for group g's blocks into tloc{A,B},
                then DMA to the contribution buffer + AllGather into the
                parity table for layer lidx."""
                b0 = 0 if g == 0 else NA_BLK
                b1 = NA_BLK if g == 0 else NB
                nb = b1 - b0
                n64 = nb * H
                sl = slice(b0 * H, b1 * H)
                tloc = tlocA if g == 0 else tlocB
                if initial:
                    tm = wk.tile([128, NA_BLK * H], f32, tag="hcb")
                    nc.vector.tensor_scalar_max(
                        out=tm[:, :n64], in0=r_sb[:, sl], scalar1=0.0
                    )
                    tm_ap = tm[:, :n64]
                else:
                    tm_ap = r_sb[:, sl]
                tp1 = wk.tile([128, NA_BLK * H], f16, tag="tp1")
                nc.scalar.activation(
                    out=tp1[:, :n64],
                    in_=tm_ap,
                    func=AF.Exp,
                    bias=tbi_sb[:, lidx : lidx + 1],
                    scale=tsc_sb[:, lidx : lidx + 1],
                )
                tme = wk.tile([128, NA_BLK * H], f16, tag="tme")
                nc.vector.tensor_scalar_add(
                    out=tme[:, :n64], in0=tm_ap, scalar1=GEN_EPS
                )
                tp2 = wk.tile([128, NA_BLK * H], f16, tag="tp2")
                nc.vector.tensor_tensor(
                    out=tp2[:, :n64], in0=tp1[:, :n64], in1=tme[:, :n64], op=OP.mult
                )
                tv = tloc[:].rearrange("p (b f) -> p b f", f=H2)
                nc.vector.tensor_copy(
                    out=tv[:, :, 0:H],
                    in_=tp2[:, :n64].rearrange("p (b f) -> p b f", f=H),
                )
                nc.vector.tensor_copy(
                    out=tv[:, :, H:H2],
                    in_=tp1[:, :n64].rearrange("p (b f) -> p b f", f=H),
                )
                d_tin = d_TinA if g == 0 else d_TinB
                nc.sync.dma_start(out=d_tin[:], in_=tloc[:])
                nc.gpsimd.collective_compute(
                    "AllGather",
                    OP.bypass,
                    replica_groups=[list(range(NC))],
                    ins=[d_tin[:]],
                    outs=[d_tabs[g][lidx % 2][:]],
                )

            def node_phase(g, l, wide64):
                """MLP + residual + (LN64, t_chunk, push) for group g blocks."""
                b0 = 0 if g == 0 else NA_BLK
                b1 = NA_BLK if g == 0 else NB
                nb = b1 - b0
                n64 = nb * H
                n128 = nb * H2
                sl64 = slice(b0 * H, b1 * H)
                l2 = slice(l * H2, (l + 1) * H2)
                lh = slice(l * H, (l + 1) * H)

                # u = agg + r  (f16)
                uu = wk.tile([128, NA_BLK * H], f16, tag="uu")
                nc.vector.tensor_tensor(
                    out=uu[:, :n64], in0=wide64[:, sl64], in1=r_sb[:, sl64], op=OP.add
                )
                # per-block transpose + W1 matmul
                h1w = wk.tile([128, NA_BLK * H2], f16, tag="h1w")
                for i in range(nb):
                    ps_t = pq.tile([H, 128], f16, tag="ptr")
                    nc.tensor.transpose(
                        out=ps_t[:], in_=uu[:, i * H : (i + 1) * H], identity=ident[:]
                    )
                    uT = ws.tile([H, 128], f16, tag="uT")
                    nc.scalar.copy(out=uT[:], in_=ps_t[:])
                    ps1 = pq.tile([128, H2], f32, tag="ph1")
                    nc.tensor.matmul(
                        out=ps1[:], lhsT=uT[:], rhs=w1_sb[:], start=True, stop=True
                    )
                    nc.scalar.copy(out=h1w[:, i * H2 : (i + 1) * H2], in_=ps1[:])
                h1v = h1w[:, :n128].rearrange("p (b f) -> p b f", f=H2)
                if l < 4:
                    # LayerNorm over 128 features, all blocks at once
                    s1 = wk.tile([128, NA_BLK], f32, tag="s1")
                    nc.vector.reduce_sum(out=s1[:, :nb], in_=h1v, axis=AX.X)
                    nmu = wk.tile([128, NA_BLK], f32, tag="nmu")
                    nc.vector.tensor_scalar_mul(
                        out=nmu[:, :nb], in0=s1[:, :nb], scalar1=-1.0 / H2
                    )
                    hc = wk.tile([128, NA_BLK * H2], f16, tag="hc")
                    hcv = hc[:, :n128].rearrange("p (b f) -> p b f", f=H2)
                    nc.vector.tensor_tensor(
                        out=hcv,
                        in0=h1v,
                        in1=nmu[:, :nb].unsqueeze(2).to_broadcast([128, nb, H2]),
                        op=OP.add,
                    )
                    nc.vector.tensor_tensor(
                        out=hcv,
                        in0=hcv,
                        in1=b1c_sb[:, l2].unsqueeze(1).to_broadcast([128, nb, H2]),
                        op=OP.add,
                    )
                    sq = wk.tile([128, NA_BLK * H2], f16, tag="sq")
                    nc.vector.tensor_tensor(
                        out=sq[:, :n128], in0=hc[:, :n128], in1=hc[:, :n128],
                        op=OP.mult,
                    )
                    s2 = wk.tile([128, NA_BLK], f32, tag="s2")
                    nc.vector.reduce_sum(
                        out=s2[:, :nb],
                        in_=sq[:, :n128].rearrange("p (b f) -> p b f", f=H2),
                        axis=AX.X,
                    )
                    sd = wk.tile([128, NA_BLK], f32, tag="sd")
                    nc.scalar.activation(
                        out=sd[:, :nb], in_=s2[:, :nb], func=AF.Sqrt,
                        bias=lneps_sb[:], scale=1.0 / H2,
                    )
                    rstd = wk.tile([128, NA_BLK], f32, tag="rstd")
                    nc.vector.reciprocal(out=rstd[:, :nb], in_=sd[:, :nb])
                    hn = wk.tile([128, NA_BLK * H2], f16, tag="hn")
                    hnv = hn[:, :n128].rearrange("p (b f) -> p b f", f=H2)
                    nc.vector.tensor_tensor(
                        out=hnv,
                        in0=hcv,
                        in1=rstd[:, :nb].unsqueeze(2).to_broadcast([128, nb, H2]),
                        op=OP.mult,
                    )
                else:
                    hn = h1w
                    hnv = h1v
                hg = wk.tile([128, NA_BLK * H2], f16, tag="sq")
                hgv = hg[:, :n128].rearrange("p (b f) -> p b f", f=H2)
                nc.vector.tensor_tensor(
                    out=hgv,
                    in0=hnv,
                    in1=gar_sb[:, l2].unsqueeze(1).to_broadcast([128, nb, H2]),
                    op=OP.mult,
                )
                nc.vector.tensor_tensor(
                    out=hgv,
                    in0=hgv,
                    in1=bar_sb[:, l2].unsqueeze(1).to_broadcast([128, nb, H2]),
                    op=OP.add,
                )
                r1 = wk.tile([128, NA_BLK * H2], f16, tag="hc")
                nc.vector.tensor_scalar_max(
                    out=r1[:, :n128], in0=hg[:, :n128], scalar1=0.0
                )
                # per-block transpose + W2 matmul -> co (reuse wide64 slices)
                for i in range(nb):
                    ps_t2 = pq.tile([128, 128], f16, tag="ptr2")
                    nc.tensor.transpose(
                        out=ps_t2[:], in_=r1[:, i * H2 : (i + 1) * H2],
                        identity=ident[:],
                    )
                    r1T = ws.tile([128, 128], f16, tag="r1T")
                    nc.scalar.copy(out=r1T[:], in_=ps_t2[:])
                    ps2 = pq.tile([128, H], f32, tag="ph2")
                    nc.tensor.matmul(
                        out=ps2[:], lhsT=r1T[:], rhs=w2_sb[:], start=True, stop=True
                    )
                    nc.scalar.copy(
                        out=wide64[:, (b0 + i) * H : (b0 + i + 1) * H], in_=ps2[:]
                    )
                co_v = wide64[:, sl64].rearrange("p (b f) -> p b f", f=H)
                b2b = b2r_sb[:, lh].unsqueeze(1).to_broadcast([128, nb, H])
                if l == 0:
                    nc.vector.tensor_tensor(
                        out=h_nxt[:, sl64].rearrange("p (b f) -> p b f", f=H),
                        in0=co_v, in1=b2b, op=OP.add,
                    )
                elif l < 4:
                    nc.vector.tensor_tensor(out=co_v, in0=co_v, in1=b2b, op=OP.add)
                    nc.vector.tensor_tensor(
                        out=h_nxt[:, sl64], in0=wide64[:, sl64], in1=h_cur[:, sl64],
                        op=OP.add,
                    )
                else:
                    nc.vector.tensor_tensor(
                        out=h_nxt[:, sl64].rearrange("p (b f) -> p b f", f=H),
                        in0=co_v, in1=b2b, op=OP.add,
                    )
                    return  # final layer: h_nxt holds the output
                # LN64 (block norm for next conv) + relu -> r_sb
                hv = h_nxt[:, sl64].rearrange("p (b f) -> p b f", f=H)
                s1b = wk.tile([128, NA_BLK], f32, tag="s1b")
                nc.vector.reduce_sum(out=s1b[:, :nb], in_=hv, axis=AX.X)
                nmub = wk.tile([128, NA_BLK], f32, tag="nmub")
                nc.vector.tensor_scalar_mul(
                    out=nmub[:, :nb], in0=s1b[:, :nb], scalar1=-1.0 / H
                )
                hcb = wk.tile([128, NA_BLK * H], f32, tag="hcb")
                hcbv = hcb[:, :n64].rearrange("p (b f) -> p b f", f=H)
                nc.vector.tensor_tensor(
                    out=hcbv,
                    in0=hv,
                    in1=nmub[:, :nb].unsqueeze(2).to_broadcast([128, nb, H]),
                    op=OP.add,
                )
                sqb = wk.tile([128, NA_BLK * H], f32, tag="sqb")
                nc.vector.tensor_tensor(
                    out=sqb[:, :n64], in0=hcb[:, :n64], in1=hcb[:, :n64], op=OP.mult
                )
                s2b = wk.tile([128, NA_BLK], f32, tag="s2b")
                nc.vector.reduce_sum(
                    out=s2b[:, :nb],
                    in_=sqb[:, :n64].rearrange("p (b f) -> p b f", f=H),
                    axis=AX.X,
                )
                sdb = wk.tile([128, NA_BLK], f32, tag="sdb")
                nc.scalar.activation(
                    out=sdb[:, :nb], in_=s2b[:, :nb], func=AF.Sqrt,
                    bias=lneps_sb[:], scale=1.0 / H,
                )
                rstdb = wk.tile([128, NA_BLK], f32, tag="rstdb")
                nc.vector.reciprocal(out=rstdb[:, :nb], in_=sdb[:, :nb])
                gsl = slice(l * H, (l + 1) * H)
                hnb = wk.tile([128, NA_BLK * H], f32, tag="sqb")
                hnbv = hnb[:, :n64].rearrange("p (b f) -> p b f", f=H)
                nc.vector.tensor_tensor(
                    out=hnbv,
                    in0=hcbv,
                    in1=rstdb[:, :nb].unsqueeze(2).to_broadcast([128, nb, H]),
                    op=OP.mult,
                )
                nc.vector.tensor_tensor(
                    out=hnbv,
                    in0=hnbv,
                    in1=blkg_sb[:, gsl].unsqueeze(1).to_broadcast([128, nb, H]),
                    op=OP.mult,
                )
                nc.vector.tensor_tensor(
                    out=hnbv,
                    in0=hnbv,
                    in1=blkb_sb[:, gsl].unsqueeze(1).to_broadcast([128, nb, H]),
                    op=OP.add,
                )
                nc.vector.tensor_scalar_max(
                    out=r_sb[:, sl64], in0=hnb[:, :n64], scalar1=0.0
                )
                # t-table for the next conv layer + push
                t_chunk_group(g, l + 1, initial=False)

            # initial t tables (layer 0) from encoder output
            t_chunk_group(0, 0, initial=True)
            t_chunk_group(1, 0, initial=True)

            NLAYERS = 5
            for l in range(NLAYERS):
                w1_sb = ws.tile([H, H2], f16, tag="w1")
                nc.sync.dma_start(out=w1_sb[:], in_=d_w1[l])
                w2_sb = ws.tile([H2, H], f16, tag="w2")
                nc.sync.dma_start(out=w2_sb[:], in_=d_w2[l])

                par = l % 2
                gbufs = {}
                qstate = [0]

                GMAX = int(os.environ.get("GMAX_TILES", "8"))

                def issue_gather(g, ci):
                    (b0_, b1_, ct0, ctn) = chunks[g][ci]
                    pool = gpa if g == 0 else gpb
                    gt = pool.tile([128, TPG * H2], f16, tag=f"g{g}")
                    for s0 in range(0, ctn, GMAX):
                        sn = min(GMAX, ctn - s0)
                        nc.gpsimd.reg_mov(nidx_reg, sn * 128)
                        nc.gpsimd.dma_gather(
                            gt[:, s0 * H2 : (s0 + sn) * H2].rearrange(
                                "p (t f) -> p t f", f=H2
                            ),
                            d_tabs[g][par][:],
                            idx_sb[:, (ct0 + s0) * 8 : (ct0 + s0 + sn) * 8],
                            sn * 128,
                            nidx_reg,
                            H2,
                            queue_num=0,
                        )
                    gbufs[(g, ci)] = (gt, ct0)

                issue_gather(0, 0)
                issue_gather(0, 1)
                issue_gather(1, 0)
                issue_gather(1, 1)

                wide64 = wd.tile([128, NBH], f32, tag="w64")

                for b in range(NB):
                    if b % CHUNK == 0 and b > 0:
                        k = b // CHUNK + 1
                        if k < NCH:
                            issue_gather(0, k)
                            issue_gather(1, k)
                    ps_agg = pp.tile([128, H2], f32, tag="pagg")
                    first = True
                    runs = []
                    for g in range(2):
                        ci = b // CHUNK
                        gt, ct0 = gbufs[(g, ci)]
                        t0 = int(gt0[g][b])
                        ntb = int(nt[g][b])
                        runs.append((g, gt, ct0, t0, ntb))
                    tot_tiles = sum(r[4] for r in runs)
                    done = 0
                    for (g, gt, ct0, t0, ntb) in runs:
                        # wide one-hot build for this block's tiles in group g
                        oh = ohp.tile([128, MAXNT * 128], f16, tag="oh")
                        nc.vector.tensor_tensor(
                            out=oh[:, : ntb * 128].rearrange(
                                "p (t f) -> p t f", f=128
                            ),
                            in0=iota_sb[:, : ntb * 128].rearrange(
                                "p (t f) -> p t f", f=128
                            ),
                            in1=drl_sb[:, t0 : t0 + ntb]
                            .unsqueeze(2)
                            .to_broadcast([128, ntb, 128]),
                            op=OP.is_equal,
                        )
                        for t in range(ntb):
                            sl_g = slice((t0 - ct0 + t) * H2, (t0 - ct0 + t + 1) * H2)
                            done += 1
                            nc.tensor.matmul(
                                out=ps_agg[:],
                                lhsT=oh[:, t * 128 : (t + 1) * 128],
                                rhs=gt[:, sl_g],
                                start=first,
                                stop=(done == tot_tiles),
                            )
                            first = False
                    den = ws.tile([128, H], f32, tag="den")
                    nc.vector.tensor_scalar_max(
                        out=den[:], in0=ps_agg[:, H:H2], scalar1=1e-16
                    )
                    rec = ws.tile([128, H], f32, tag="rec")
                    nc.vector.reciprocal(out=rec[:], in_=den[:])
                    nc.vector.tensor_tensor(
                        out=wide64[:, b * H : (b + 1) * H],
                        in0=ps_agg[:, 0:H],
                        in1=rec[:],
                        op=OP.mult,
                    )
                    if b == NA_BLK - 1:
                        node_phase(0, l, wide64)
                node_phase(1, l, wide64)
                if l < 4:
                    h_cur, h_nxt = h_nxt, h_cur

            # h_nxt (not swapped after l=4) holds the final output
            out_final = h_nxt if NLAYERS in (0, 1, 5) else h_cur
            if NLAYERS == 0:
                out_final = r_sb
            nc.sync.dma_start(
                out=d_out[:].rearrange("(b p) f -> p b f", p=128),
                in_=out_final[:].rearrange("p (b f) -> p b f", f=H),
            )

    lower_extended_insts(nc)

    in_maps = []
    for cc in range(NC):
        in_maps.append(
            {
                "xT": np.ascontiguousarray(xT[cc]),
                "idx16": np.ascontiguousarray(idx16[cc]),
                "drl": np.ascontiguousarray(drl[cc]),
                "w1all": w1all,
                "w2all": w2all,
                "b1c": b1c,
                "garr": garr,
                "barr": barr,
                "b2r": b2r,
                "blkg": blkg,
                "blkb": blkb,
                "tsc": tsc,
                "tbi": tbi,
                "lneps": np.full((128, 1), LN_EPS, np.float32),
                "iota": iota_rep,
                "ident": np.eye(128, dtype=np.float16),
                "encW": encW,
                "encb": encb,
            }
        )
    res = run_bass_kernel_spmd(nc, in_maps, list(range(NC)), trace=_trace)
    LAST_EXEC_NS = res.exec_time_ns
    out = np.empty((N, H), dtype=np.float32)
    for cc in range(NC):
        oc = res.results[cc]["out"]
        out[cc * NPC : (cc + 1) * NPC] = oc[:NPC]
    return out.astype(np.float32)


# revision 3
# speedup vs baseline: 2.0635x; 2.0635x over previous
"""DeeperGCN forward on 8 Trainium2 NeuronCores (Bass/Tile) — v2.

Strategy (dst-node sharding, batched SWDGE gathers):
- 6250 nodes/core in 49 fixed blocks of 128 (last 106). Per layer each core
  computes its nodes' [P2|P1] = [msg*exp(t*msg) | exp(t*msg)] rows (f16,
  256B), split into group A (blocks 0..24) and B (25..48); AllGather builds
  two replicated tables tabA [25600,128] / tabB [24576,128] — each small
  enough for int16 dma_gather row indices.
- Edge phase: edges live at their dst core, grouped by (src-group, chunk of
  4 dst blocks). One dma_gather per (group, chunk) fetches ~36 tiles of 128
  src rows in a single SWDGE instruction (amortizes the ~1us fixed
  descriptor-gen cost that dominated v1). Aggregation per dst block is
  one-hot matmuls accumulated in PSUM: agg = num/max(den,1e-16) reproduces
  the softmax aggregation exactly.
- Node phase runs layer-wide per group with wide (multi-block) DVE/ACT ops:
  LN via segmented reduces over 3D views, single Exp/Sqrt activations per
  group, per-block PE transposes + f16 matmuls for the MLP.
"""

import json
import os
import sys
import types

import numpy as np

sys.path.insert(0, "/opt/trn_rl_repo")

# ---------------------------------------------------------------------------
# Workaround: this walrus build supports only ONE semaphore wait per
# instruction; Tile attaches several. Split extras onto NoOp instructions
# at BIR-JSON serialization time.
# ---------------------------------------------------------------------------
_PATCHED = False


def _install_bir_patch():
    global _PATCHED
    if _PATCHED:
        return
    _PATCHED = True
    import concourse.bass as bass

    orig = bass.Bass.to_json_bytes

    def patched(self):
        data = json.loads(orig(self).decode())
        ctr = 0
        for fn in data.get("functions", []):
            for bb in fn.get("blocks", []):
                new_insts = []
                for inst in bb.get("instructions", []):
                    si = inst.get("sync_info")
                    waits = (si or {}).get("on_wait") or []
                    if len(waits) > 1:
                        for w in waits[:-1]:
                            ctr += 1
                            nop = {
                                "engine": inst["engine"],
                                "ins": [],
                                "outs": [],
                                "name": f"{inst['name']}-sw{ctr}",
                                "opcode": "NoOp",
                                "sync_info": {"on_update": [], "on_wait": [w]},
                            }
                            if "debug" in inst:
                                nop["debug"] = inst["debug"]
                            new_insts.append(nop)
                        si["on_wait"] = [waits[-1]]
                    new_insts.append(inst)
                bb["instructions"] = new_insts
        return json.dumps(data).encode()

    bass.Bass.to_json_bytes = patched


def _install_trace_hook():
    """Optional: register the NTFF profiling hook (for test.py timing)."""
    import antenv

    if "antenv.axon_hooks" in sys.modules:
        return
    _m = types.ModuleType("antenv.axon_hooks")
    _m._hook = None
    _m.set_axon_ntff_profile_hook = lambda h: setattr(_m, "_hook", h)
    _m.get_axon_ntff_profile_hook = lambda: _m._hook
    sys.modules["antenv.axon_hooks"] = _m
    antenv.axon_hooks = _m
    try:
        from trn_agent_boot.trn_boot import _ntff_profile_via_ctypes

        _m._hook = _ntff_profile_via_ctypes("/opt/axon/libaxon_pjrt.so")
    except Exception:
        pass


N, NC, NPC = 50000, 8, 6250
H = 64
H2 = 128
F_IN = 128
LN_EPS = 1e-5
BN_EPS = 1e-5
GEN_EPS = 1e-7

NB = 49          # dst blocks per core (128 nodes, last has 106)
NA_BLK = 25      # group A: blocks 0..24
RA = NA_BLK * 128       # 3200 rows/core in table A
RB = (NB - NA_BLK) * 128  # 3072 rows/core in table B
TABA = NC * RA   # 25600
TABB = NC * RB   # 24576
CHUNK = 2        # dst blocks per gather chunk
NCOLS = NB * 128  # 6272

LAST_EXEC_NS = None


def _preprocess_edges(edge_index):
    """Cross-core-uniform tile geometry + per-core gather index / dst-label
    arrays.

    Returns (nt, gt0, chunks, TOT, MAXNT, idx16, drl) where
      nt[g][b]   tiles for (group g, dst block b)        (uniform)
      gt0[g][b]  global tile index of first tile of (g,b)
      chunks[g]  list of (b_lo, b_hi, ct0, ctiles) per gather chunk
      idx16      [NC, 128, TOT*8] int16 gather indices (16-wrapped, replicated)
      drl        [NC, 128, TOT] f16 dst position labels (255 = pad)
    """
    src = np.asarray(edge_index[0], dtype=np.int64)
    dst = np.asarray(edge_index[1], dtype=np.int64)
    c = dst // NPC
    ld = dst - c * NPC
    db = ld >> 7
    dp = ld & 127
    cs = src // NPC
    ls = src - cs * NPC
    bs = ls >> 7
    ps = ls & 127
    grp = (bs >= NA_BLK).astype(np.int64)
    row = np.where(
        grp == 0,
        cs * RA + ps * NA_BLK + bs,
        cs * RB + ps * (NB - NA_BLK) + (bs - NA_BLK),
    ).astype(np.int64)

    # per (core, grp, block) edge counts -> uniform tile counts
    key = (c * 2 + grp) * NB + db
    cnt = np.bincount(key, minlength=NC * 2 * NB).reshape(NC, 2, NB)
    nt = np.maximum(1, -(-cnt.max(axis=0) // 128))  # [2, NB] cross-core max

    # chunk partition of blocks (same for both groups)
    blos = list(range(0, NB, CHUNK))
    chunk_ranges = [(b0, min(b0 + CHUNK, NB)) for b0 in blos]

    # global tile numbering: group 0 chunks then group 1 chunks
    gt0 = np.zeros((2, NB), dtype=np.int64)
    chunks = [[], []]
    t = 0
    for g in range(2):
        for (b0, b1) in chunk_ranges:
            ct0 = t
            for b in range(b0, b1):
                gt0[g, b] = t
                t += int(nt[g, b])
            chunks[g].append((b0, b1, ct0, t - ct0))
    TOT = t
    MAXNT = int(nt.max())

    # per-edge flat slot: gt0[g,db]*128 + rank within (c,g,db)
    order = np.lexsort((db, grp, c))
    inv = np.empty_like(order)
    inv[order] = np.arange(len(order))
    # rank within each (c,g,db) bucket
    sorted_key = key[order]
    starts = np.searchsorted(sorted_key, np.arange(NC * 2 * NB), side="left")
    rank_sorted = np.arange(len(order)) - starts[sorted_key]
    rank = np.empty_like(rank_sorted)
    rank[order] = rank_sorted

    slot = gt0[grp, db] * 128 + rank  # within-core flat element slot

    idx16 = np.zeros((NC, 16, TOT * 8), dtype=np.int16)
    drl = np.full((NC, 128, TOT), 255.0, dtype=np.float16)
    for cc in range(NC):
        m = c == cc
        fl_idx = np.zeros(TOT * 128, dtype=np.int16)
        fl_drl = np.full(TOT * 128, 255, dtype=np.int64)
        fl_idx[slot[m]] = row[m].astype(np.int16)
        fl_drl[slot[m]] = dp[m]
        # idx wrap: element i -> [i%16, i//16]
        idx16[cc] = fl_idx.reshape(TOT * 8, 16).T
        # drl: element i of tile t -> [i%128, t]
        drl[cc] = fl_drl.reshape(TOT, 128).T.astype(np.float16)
    idx16 = np.tile(idx16, (1, 8, 1))  # replicate to 128 partitions
    return nt, gt0, chunks, TOT, MAXNT, idx16, drl


def kernel(
    x,
    edge_index,
    enc_W,
    enc_b,
    conv_t,
    conv_W1,
    conv_b1,
    conv_lng,
    conv_lnb,
    conv_W2,
    conv_b2,
    block_lng,
    block_lnb,
    fin_t,
    fin_W1,
    fin_b1,
    fin_bng,
    fin_bnb,
    fin_W2,
    fin_b2,
    _trace=False,
):
    global LAST_EXEC_NS
    _install_bir_patch()
    if _trace:
        _install_trace_hook()

    import concourse.bass as bass
    import concourse.mybir as mybir
    import concourse.tile as tile
    from concourse import library_config
    from concourse.bass_utils import run_bass_kernel_spmd
    from concourse.library_overlay import lower_extended_insts
    f32 = mybir.dt.float32
    f16 = mybir.dt.float16
    i16 = mybir.dt.int16
    AF = mybir.ActivationFunctionType
    OP = mybir.AluOpType
    AX = mybir.AxisListType

    x = np.asarray(x, dtype=np.float32)
    nt, gt0, chunks, TOT, MAXNT, idx16, drl = _preprocess_edges(
        np.asarray(edge_index)
    )
    NCH = len(chunks[0])  # chunks per group

    # ---------------- host-side parameter prep (replicated) ----------------
    rep = lambda v, w: np.ascontiguousarray(
        np.broadcast_to(np.asarray(v, np.float32).reshape(1, w), (128, w))
    )
    w1all = np.concatenate(
        [np.asarray(conv_W1, np.float32), np.asarray(fin_W1, np.float32)[None]], 0
    ).astype(np.float16)  # [5, 64, 128]
    w2all = np.concatenate(
        [np.asarray(conv_W2, np.float32), np.asarray(fin_W2, np.float32)[None]], 0
    ).astype(np.float16)  # [5, 128, 64]
    b1 = np.asarray(conv_b1, np.float32)
    b1c_list = [b1[i] - b1[i].mean() for i in range(4)] + [np.zeros(H2, np.float32)]
    b1c = np.concatenate([rep(v, H2) for v in b1c_list], axis=1)  # [128, 5*128]
    g_fin = np.asarray(fin_bng, np.float32) / np.sqrt(np.float32(1.0 + BN_EPS))
    garr = np.concatenate(
        [rep(v, H2) for v in list(np.asarray(conv_lng, np.float32)) + [g_fin]], axis=1
    )
    bar_fin = np.asarray(fin_b1, np.float32) * g_fin + np.asarray(fin_bnb, np.float32)
    barr = np.concatenate(
        [rep(v, H2) for v in list(np.asarray(conv_lnb, np.float32)) + [bar_fin]],
        axis=1,
    )
    b2r = np.concatenate(
        [rep(v, H) for v in list(np.asarray(conv_b2, np.float32)) + [np.asarray(fin_b2)]],
        axis=1,
    )  # [128, 5*64]
    blg = np.asarray(block_lng, np.float32)
    blb = np.asarray(block_lnb, np.float32)
    blkg = np.concatenate([rep(blg[i], H) for i in (1, 2, 3, 0)], axis=1)
    blkb = np.concatenate([rep(blb[i], H) for i in (1, 2, 3, 0)], axis=1)
    tvals = np.array(
        list(np.asarray(conv_t, np.float32)) + [float(np.asarray(fin_t))], np.float32
    )
    tsc = np.ascontiguousarray(np.broadcast_to(tvals.reshape(1, 5), (128, 5)))
    tbi = np.ascontiguousarray(tsc * np.float32(GEN_EPS))
    iota_rep = np.tile(
        np.arange(128, dtype=np.float16), (128, MAXNT)
    ).reshape(128, MAXNT * 128)
    encW = np.asarray(enc_W, np.float32).astype(np.float16)  # [128, 64]
    encb = rep(enc_b, H)

    # per-core transposed x: xT[c][f, b*128+p] = x[c*NPC + b*128 + p, f]
    xT = np.zeros((NC, 128, NCOLS), dtype=np.float16)
    for cc in range(NC):
        xc = x[cc * NPC : (cc + 1) * NPC]  # [6250, 128]
        xT[cc, :, : xc.shape[0]] = 0
        full = np.zeros((NCOLS, 128), np.float32)
        full[: xc.shape[0]] = xc
        xT[cc] = full.T.astype(np.float16)

    # ---------------- build the Bass program ----------------
    nc = bass.Bass(num_swdge_queues=4, dynamic_dma_scratch_size=32768)

    d_xT = nc.dram_tensor("xT", [128, NCOLS], f16, kind="ExternalInput")
    d_idx = nc.dram_tensor("idx16", [128, TOT * 8], i16, kind="ExternalInput")
    d_drl = nc.dram_tensor("drl", [128, TOT], f16, kind="ExternalInput")
    d_w1 = nc.dram_tensor("w1all", [5, H, H2], f16, kind="ExternalInput")
    d_w2 = nc.dram_tensor("w2all", [5, H2, H], f16, kind="ExternalInput")
    d_b1c = nc.dram_tensor("b1c", [128, 5 * H2], f32, kind="ExternalInput")
    d_gar = nc.dram_tensor("garr", [128, 5 * H2], f32, kind="ExternalInput")
    d_bar = nc.dram_tensor("barr", [128, 5 * H2], f32, kind="ExternalInput")
    d_b2r = nc.dram_tensor("b2r", [128, 5 * H], f32, kind="ExternalInput")
    d_blkg = nc.dram_tensor("blkg", [128, 4 * H], f32, kind="ExternalInput")
    d_blkb = nc.dram_tensor("blkb", [128, 4 * H], f32, kind="ExternalInput")
    d_tsc = nc.dram_tensor("tsc", [128, 5], f32, kind="ExternalInput")
    d_tbi = nc.dram_tensor("tbi", [128, 5], f32, kind="ExternalInput")
    d_lneps = nc.dram_tensor("lneps", [128, 1], f32, kind="ExternalInput")
    d_iota = nc.dram_tensor("iota", [128, MAXNT * 128], f16, kind="ExternalInput")
    d_ident = nc.dram_tensor("ident", [128, 128], f16, kind="ExternalInput")
    d_encW = nc.dram_tensor("encW", [128, H], f16, kind="ExternalInput")
    d_encb = nc.dram_tensor("encb", [128, H], f32, kind="ExternalInput")
    d_out = nc.dram_tensor("out", [NCOLS, H], f32, kind="ExternalOutput")

    d_TinA = nc.dram_tensor("T_in_a", [128, RA], f16)
    d_TinB = nc.dram_tensor("T_in_b", [128, RB], f16)
    d_tabA = [
        nc.dram_tensor(f"T_tabA{i}", [TABA, H2], f16, addr_space="Shared")
        for i in range(2)
    ]
    d_tabB = [
        nc.dram_tensor(f"T_tabB{i}", [TABB, H2], f16, addr_space="Shared")
        for i in range(2)
    ]
    d_tabs = [d_tabA, d_tabB]

    NBH = NB * H  # 3136

    # max tiles in one gather chunk
    TPG = max(int(ct) for g in range(2) for (_, _, _, ct) in chunks[g])

    with tile.TileContext(nc) as tc:
        nc.gpsimd.load_library(library_config.mlp)
        nidx_reg = nc.gpsimd.alloc_register(name="nidx")
        with (
            tc.tile_pool(name="state", bufs=1) as st,
            tc.tile_pool(name="wkw", bufs=1) as wk,
            tc.tile_pool(name="wks", bufs=2) as ws,
            tc.tile_pool(name="wide", bufs=1) as wd,
            tc.tile_pool(name="ga", bufs=4) as gpa,
            tc.tile_pool(name="gb", bufs=4) as gpb,
            tc.tile_pool(name="ohp", bufs=1) as ohp,
            tc.tile_pool(name="psa", bufs=3, space="PSUM") as pp,
            tc.tile_pool(name="psq", bufs=1, space="PSUM") as pq,
        ):
            # ---------------- persistent state / constants ----------------
            idx_sb = st.tile([128, TOT * 8], i16, tag="idx")
            nc.sync.dma_start(out=idx_sb[:], in_=d_idx[:])
            drl_sb = st.tile([128, TOT], f16, tag="drl")
            nc.sync.dma_start(out=drl_sb[:], in_=d_drl[:])
            iota_sb = st.tile([128, MAXNT * 128], f16, tag="iota")
            nc.sync.dma_start(out=iota_sb[:], in_=d_iota[:])
            ident = st.tile([128, 128], f16, tag="ident")
            nc.sync.dma_start(out=ident[:], in_=d_ident[:])
            b1c_sb = st.tile([128, 5 * H2], f32, tag="b1c")
            nc.sync.dma_start(out=b1c_sb[:], in_=d_b1c[:])
            gar_sb = st.tile([128, 5 * H2], f32, tag="gar")
            nc.sync.dma_start(out=gar_sb[:], in_=d_gar[:])
            bar_sb = st.tile([128, 5 * H2], f32, tag="bar")
            nc.sync.dma_start(out=bar_sb[:], in_=d_bar[:])
            b2r_sb = st.tile([128, 5 * H], f32, tag="b2r")
            nc.sync.dma_start(out=b2r_sb[:], in_=d_b2r[:])
            blkg_sb = st.tile([128, 4 * H], f32, tag="blkg")
            nc.sync.dma_start(out=blkg_sb[:], in_=d_blkg[:])
            blkb_sb = st.tile([128, 4 * H], f32, tag="blkb")
            nc.sync.dma_start(out=blkb_sb[:], in_=d_blkb[:])
            tsc_sb = st.tile([128, 5], f32, tag="tsc")
            nc.sync.dma_start(out=tsc_sb[:], in_=d_tsc[:])
            tbi_sb = st.tile([128, 5], f32, tag="tbi")
            nc.sync.dma_start(out=tbi_sb[:], in_=d_tbi[:])
            lneps_sb = st.tile([128, 1], f32, tag="lneps")
            nc.sync.dma_start(out=lneps_sb[:], in_=d_lneps[:])
            encb_sb = st.tile([128, H], f32, tag="encb")
            nc.sync.dma_start(out=encb_sb[:], in_=d_encb[:])
            encW_sb = st.tile([128, H], f16, tag="encW")
            nc.sync.dma_start(out=encW_sb[:], in_=d_encW[:])
            xT_f32 = wd.tile([128, NBH], f32, tag="w64")
            xT_sb = xT_f32[:].bitcast(f16)
            nc.sync.dma_start(out=xT_sb, in_=d_xT[:])
            tlocA = st.tile([128, RA], f16, tag="tlocA")
            tlocB = st.tile([128, RB], f16, tag="tlocB")

            h_a = st.tile([128, NBH], f32, tag="h_a")
            h_b = st.tile([128, NBH], f32, tag="h_b")
            r_sb = st.tile([128, NBH], f32, tag="r_sb")

            # ---------------- encoder: r = x @ enc_W + enc_b ----------------
            for b in range(NB):
                ps_e = pq.tile([128, H], f32, tag="pe")
                nc.tensor.matmul(
                    out=ps_e[:],
                    lhsT=xT_sb[:, b * 128 : (b + 1) * 128],
                    rhs=encW_sb[:],
                    start=True,
                    stop=True,
                )
                nc.scalar.copy(out=r_sb[:, b * H : (b + 1) * H], in_=ps_e[:])
            nc.vector.tensor_tensor(
                out=r_sb[:].rearrange("p (b f) -> p b f", f=H),
                in0=r_sb[:].rearrange("p (b f) -> p b f", f=H),
                in1=encb_sb[:].unsqueeze(1).to_broadcast([128, NB, H]),
                op=OP.add,
            )

            h_cur, h_nxt = h_a, h_b

            def t_chunk_group(g, lidx, initial):
                """Compute [P2|P1] rows for group g's blocks into tloc{A,B},
                then DMA to the contribution buffer + AllGather into the
                parity table for layer lidx."""
                b0 = 0 if g == 0 else NA_BLK
                b1 = NA_BLK if g == 0 else NB
                nb = b1 - b0
                n64 = nb * H
                sl = slice(b0 * H, b1 * H)
                tloc = tlocA if g == 0 else tlocB
                if initial:
                    tm = wk.tile([128, NA_BLK * H], f32, tag="hcb")
                    nc.vector.tensor_scalar_max(
                        out=tm[:, :n64], in0=r_sb[:, sl], scalar1=0.0
                    )
                    tm_ap = tm[:, :n64]
                else:
                    tm_ap = r_sb[:, sl]
                tp1 = wk.tile([128, NA_BLK * H], f16, tag="tp1")
                nc.scalar.activation(
                    out=tp1[:, :n64],
                    in_=tm_ap,
                    func=AF.Exp,
                    bias=tbi_sb[:, lidx : lidx + 1],
                    scale=tsc_sb[:, lidx : lidx + 1],
                )
                tme = wk.tile([128, NA_BLK * H], f16, tag="tme")
                nc.vector.tensor_scalar_add(
                    out=tme[:, :n64], in0=tm_ap, scalar1=GEN_EPS
                )
                tp2 = wk.tile([128, NA_BLK * H], f16, tag="tp2")
                nc.vector.tensor_tensor(
                    out=tp2[:, :n64], in0=tp1[:, :n64], in1=tme[:, :n64], op=OP.mult
                )
                tv = tloc[:].rearrange("p (b f) -> p b f", f=H2)
                nc.vector.tensor_copy(
                    out=tv[:, :, 0:H],
                    in_=tp2[:, :n64].rearrange("p (b f) -> p b f", f=H),
                )
                nc.vector.tensor_copy(
                    out=tv[:, :, H:H2],
                    in_=tp1[:, :n64].rearrange("p (b f) -> p b f", f=H),
                )
                d_tin = d_TinA if g == 0 else d_TinB
                nc.sync.dma_start(out=d_tin[:], in_=tloc[:])
                nc.gpsimd.collective_compute(
                    "AllGather",
                    OP.bypass,
                    replica_groups=[list(range(NC))],
                    ins=[d_tin[:]],
                    outs=[d_tabs[g][lidx % 2][:]],
                )

            def node_phase(g, l, wide64):
                """MLP + residual + (LN64, t_chunk, push) for group g blocks."""
                b0 = 0 if g == 0 else NA_BLK
                b1 = NA_BLK if g == 0 else NB
                nb = b1 - b0
                n64 = nb * H
                n128 = nb * H2
                sl64 = slice(b0 * H, b1 * H)
                l2 = slice(l * H2, (l + 1) * H2)
                lh = slice(l * H, (l + 1) * H)

                # u = agg + r  (f16)
                uu = wk.tile([128, NA_BLK * H], f16, tag="uu")
                nc.vector.tensor_tensor(
                    out=uu[:, :n64], in0=wide64[:, sl64], in1=r_sb[:, sl64], op=OP.add
                )
                # per-block transpose + W1 matmul
                h1w = wk.tile([128, NA_BLK * H2], f16, tag="h1w")
                for i in range(nb):
                    ps_t = pq.tile([H, 128], f16, tag="ptr")
                    nc.tensor.transpose(
                        out=ps_t[:], in_=uu[:, i * H : (i + 1) * H], identity=ident[:]
                    )
                    uT = ws.tile([H, 128], f16, tag="uT")
                    nc.scalar.copy(out=uT[:], in_=ps_t[:])
                    ps1 = pq.tile([128, H2], f32, tag="ph1")
                    nc.tensor.matmul(
                        out=ps1[:], lhsT=uT[:], rhs=w1_sb[:], start=True, stop=True
                    )
                    nc.scalar.copy(out=h1w[:, i * H2 : (i + 1) * H2], in_=ps1[:])
                h1v = h1w[:, :n128].rearrange("p (b f) -> p b f", f=H2)
                if l < 4:
                    # LayerNorm over 128 features, all blocks at once
                    s1 = wk.tile([128, NA_BLK], f32, tag="s1")
                    nc.vector.reduce_sum(out=s1[:, :nb], in_=h1v, axis=AX.X)
                    nmu = wk.tile([128, NA_BLK], f32, tag="nmu")
                    nc.vector.tensor_scalar_mul(
                        out=nmu[:, :nb], in0=s1[:, :nb], scalar1=-1.0 / H2
                    )
                    hc = wk.tile([128, NA_BLK * H2], f16, tag="hc")
                    hcv = hc[:, :n128].rearrange("p (b f) -> p b f", f=H2)
                    nc.vector.tensor_tensor(
                        out=hcv,
                        in0=h1v,
                        in1=nmu[:, :nb].unsqueeze(2).to_broadcast([128, nb, H2]),
                        op=OP.add,
                    )
                    nc.vector.tensor_tensor(
                        out=hcv,
                        in0=hcv,
                        in1=b1c_sb[:, l2].unsqueeze(1).to_broadcast([128, nb, H2]),
                        op=OP.add,
                    )
                    sq = wk.tile([128, NA_BLK * H2], f16, tag="sq")
                    nc.vector.tensor_tensor(
                        out=sq[:, :n128], in0=hc[:, :n128], in1=hc[:, :n128],
                        op=OP.mult,
                    )
                    s2 = wk.tile([128, NA_BLK], f32, tag="s2")
                    nc.vector.reduce_sum(
                        out=s2[:, :nb],
                        in_=sq[:, :n128].rearrange("p (b f) -> p b f", f=H2),
                        axis=AX.X,
                    )
                    sd = wk.tile([128, NA_BLK], f32, tag="sd")
                    nc.scalar.activation(
                        out=sd[:, :nb], in_=s2[:, :nb], func=AF.Sqrt,
                        bias=lneps_sb[:], scale=1.0 / H2,
                    )
                    rstd = wk.tile([128, NA_BLK], f32, tag="rstd")
                    nc.vector.reciprocal(out=rstd[:, :nb], in_=sd[:, :nb])
                    hn = wk.tile([128, NA_BLK * H2], f16, tag="hn")
                    hnv = hn[:, :n128].rearrange("p (b f) -> p b f", f=H2)
                    nc.vector.tensor_tensor(
                        out=hnv,
                        in0=hcv,
                        in1=rstd[:, :nb].unsqueeze(2).to_broadcast([128, nb, H2]),
                        op=OP.mult,
                    )
                else:
                    hn = h1w
                    hnv = h1v
                hg = wk.tile([128, NA_BLK * H2], f16, tag="sq")
                hgv = hg[:, :n128].rearrange("p (b f) -> p b f", f=H2)
                nc.vector.tensor_tensor(
                    out=hgv,
                    in0=hnv,
                    in1=gar_sb[:, l2].unsqueeze(1).to_broadcast([128, nb, H2]),
                    op=OP.mult,
                )
                nc.vector.tensor_tensor(
                    out=hgv,
                    in0=hgv,
                    in1=bar_sb[:, l2].unsqueeze(1).to_broadcast([128, nb, H2]),
                    op=OP.add,
                )
                r1 = wk.tile([128, NA_BLK * H2], f16, tag="hc")
                nc.vector.tensor_scalar_max(
                    out=r1[:, :n128], in0=hg[:, :n128], scalar1=0.0
                )
                # per-block transpose + W2 matmul -> co (reuse wide64 slices)
                for i in range(nb):
                    ps_t2 = pq.tile([128, 128], f16, tag="ptr2")
                    nc.tensor.transpose(
                        out=ps_t2[:], in_=r1[:, i * H2 : (i + 1) * H2],
                        identity=ident[:],
                    )
                    r1T = ws.tile([128, 128], f16, tag="r1T")
                    nc.scalar.copy(out=r1T[:], in_=ps_t2[:])
                    ps2 = pq.tile([128, H], f32, tag="ph2")
                    nc.tensor.matmul(
                        out=ps2[:], lhsT=r1T[:], rhs=w2_sb[:], start=True, stop=True
                    )
                    nc.scalar.copy(
                        out=wide64[:, (b0 + i) * H : (b0 + i + 1) * H], in_=ps2[:]
                    )
                co_v = wide64[:, sl64].rearrange("p (b f) -> p b f", f=H)
                b2b = b2r_sb[:, lh].unsqueeze(1).to_broadcast([128, nb, H])
                if l == 0:
                    nc.vector.tensor_tensor(
                        out=h_nxt[:, sl64].rearrange("p (b f) -> p b f", f=H),
                        in0=co_v, in1=b2b, op=OP.add,
                    )
                elif l < 4:
                    nc.vector.tensor_tensor(out=co_v, in0=co_v, in1=b2b, op=OP.add)
                    nc.vector.tensor_tensor(
                        out=h_nxt[:, sl64], in0=wide64[:, sl64], in1=h_cur[:, sl64],
                        op=OP.add,
                    )
                else:
                    nc.vector.tensor_tensor(
                        out=h_nxt[:, sl64].rearrange("p (b f) -> p b f", f=H),
                        in0=co_v, in1=b2b, op=OP.add,
                    )
                    return  # final layer: h_nxt holds the output
                # LN64 (block norm for next conv) + relu -> r_sb
                hv = h_nxt[:, sl64].rearrange("p (b f) -> p b f", f=H)
                s1b = wk.tile([128, NA_BLK], f32, tag="s1b")
                nc.vector.reduce_sum(out=s1b[:, :nb], in_=hv, axis=AX.X)
                nmub = wk.tile([128, NA_BLK], f32, tag="nmub")
                nc.vector.tensor_scalar_mul(
                    out=nmub[:, :nb], in0=s1b[:, :nb], scalar1=-1.0 / H
                )
                hcb = wk.tile([128, NA_BLK * H], f32, tag="hcb")
                hcbv = hcb[:, :n64].rearrange("p (b f) -> p b f", f=H)
                nc.vector.tensor_tensor(
                    out=hcbv,
                    in0=hv,
                    in1=nmub[:, :nb].unsqueeze(2).to_broadcast([128, nb, H]),
                    op=OP.add,
                )
                sqb = wk.tile([128, NA_BLK * H], f32, tag="sqb")
                nc.vector.tensor_tensor(
                    out=sqb[:, :n64], in0=hcb[:, :n64], in1=hcb[:, :n64], op=OP.mult
                )
                s2b = wk.tile([128, NA_BLK], f32, tag="s2b")
                nc.vector.reduce_sum(
                    out=s2b[:, :nb],
                    in_=sqb[:, :n64].rearrange("p (b f) -> p b f", f=H),
                    axis=AX.X,
                )
                sdb = wk.tile([128, NA_BLK], f32, tag="sdb")
                nc.scalar.activation(
                    out=sdb[:, :nb], in_=s2b[:, :nb], func=AF.Sqrt,
                    bias=lneps_sb[:], scale=1.0 / H,
                )
                rstdb = wk.tile([128, NA_BLK], f32, tag="rstdb")
                nc.vector.reciprocal(out=rstdb[:, :nb], in_=sdb[:, :nb])
                gsl = slice(l * H, (l + 1) * H)
                hnb = wk.tile([128, NA_BLK * H], f32, tag="sqb")
                hnbv = hnb[:, :n64].rearrange("p (b f) -> p b f", f=H)
                nc.vector.tensor_tensor(
                    out=hnbv,
                    in0=hcbv,
                    in1=rstdb[:, :nb].unsqueeze(2).to_broadcast([128, nb, H]),
                    op=OP.mult,
                )
                nc.vector.tensor_tensor(
                    out=hnbv,
                    in0=hnbv,
                    in1=blkg_sb[:, gsl].unsqueeze(1).to_broadcast([128, nb, H]),
                    op=OP.mult,
                )
                nc.vector.tensor_tensor(
                    out=hnbv,
                    in0=hnbv,
                    in1=blkb_sb[:, gsl].unsqueeze(1).to_broadcast([128, nb, H]),
                    op=OP.add,
                )
                nc.vector.tensor_scalar_max(
                    out=r_sb[:, sl64], in0=hnb[:, :n64], scalar1=0.0
                )
                # t-table for the next conv layer + push
                t_chunk_group(g, l + 1, initial=False)

            # initial t tables (layer 0) from encoder output
            t_chunk_group(0, 0, initial=True)
            t_chunk_group(1, 0, initial=True)

            NLAYERS = 5
            for l in range(NLAYERS):
                w1_sb = ws.tile([H, H2], f16, tag="w1")
                nc.sync.dma_start(out=w1_sb[:], in_=d_w1[l])
                w2_sb = ws.tile([H2, H], f16, tag="w2")
                nc.sync.dma_start(out=w2_sb[:], in_=d_w2[l])

                par = l % 2
                gbufs = {}
                qstate = [0]

                GMAX = int(os.environ.get("GMAX_TILES", "8"))

                def issue_gather(g, ci):
                    (b0_, b1_, ct0, ctn) = chunks[g][ci]
                    pool = gpa if g == 0 else gpb
                    gt = pool.tile([128, TPG * H2], f16, tag=f"g{g}")
                    for s0 in range(0, ctn, GMAX):
                        sn = min(GMAX, ctn - s0)
                        nc.gpsimd.reg_mov(nidx_reg, sn * 128)
                        nc.gpsimd.dma_gather(
                            gt[:, s0 * H2 : (s0 + sn) * H2].rearrange(
                                "p (t f) -> p t f", f=H2
                            ),
                            d_tabs[g][par][:],
                            idx_sb[:, (ct0 + s0) * 8 : (ct0 + s0 + sn) * 8],
                            sn * 128,
                            nidx_reg,
                            H2,
                            queue_num=qstate[0],
                        )
                        qstate[0] = (qstate[0] + 1) % 4
                    gbufs[(g, ci)] = (gt, ct0)

                issue_gather(0, 0)
                issue_gather(0, 1)
                issue_gather(0, 2)
                issue_gather(1, 0)
                issue_gather(1, 1)
                issue_gather(1, 2)

                wide64 = wd.tile([128, NBH], f32, tag="w64")

                for b in range(NB):
                    if b % CHUNK == 0:
                        k = b // CHUNK + 3
                        if k < NCH:
                            issue_gather(0, k)
                            issue_gather(1, k)
                    ps_agg = pp.tile([128, H2], f32, tag="pagg")
                    first = True
                    runs = []
                    for g in range(2):
                        ci = b // CHUNK
                        gt, ct0 = gbufs[(g, ci)]
                        t0 = int(gt0[g][b])
                        ntb = int(nt[g][b])
                        runs.append((g, gt, ct0, t0, ntb))
                    tot_tiles = sum(r[4] for r in runs)
                    done = 0
                    for (g, gt, ct0, t0, ntb) in runs:
                        # wide one-hot build for this block's tiles in group g
                        oh = ohp.tile([128, MAXNT * 128], f16, tag="oh")
                        nc.vector.tensor_tensor(
                            out=oh[:, : ntb * 128].rearrange(
                                "p (t f) -> p t f", f=128
                            ),
                            in0=iota_sb[:, : ntb * 128].rearrange(
                                "p (t f) -> p t f", f=128
                            ),
                            in1=drl_sb[:, t0 : t0 + ntb]
                            .unsqueeze(2)
                            .to_broadcast([128, ntb, 128]),
                            op=OP.is_equal,
                        )
                        for t in range(ntb):
                            sl_g = slice((t0 - ct0 + t) * H2, (t0 - ct0 + t + 1) * H2)
                            done += 1
                            nc.tensor.matmul(
                                out=ps_agg[:],
                                lhsT=oh[:, t * 128 : (t + 1) * 128],
                                rhs=gt[:, sl_g],
                                start=first,
                                stop=(done == tot_tiles),
                            )
                            first = False
                    den = ws.tile([128, H], f32, tag="den")
                    nc.vector.tensor_scalar_max(
                        out=den[:], in0=ps_agg[:, H:H2], scalar1=1e-16
                    )
                    rec = ws.tile([128, H], f32, tag="rec")
                    nc.vector.reciprocal(out=rec[:], in_=den[:])
                    nc.vector.tensor_tensor(
                        out=wide64[:, b * H : (b + 1) * H],
                        in0=ps_agg[:, 0:H],
                        in1=rec[:],
                        op=OP.mult,
                    )
                    if b == NA_BLK - 1:
                        node_phase(0, l, wide64)
                node_phase(1, l, wide64)
                if l < 4:
                    h_cur, h_nxt = h_nxt, h_cur

            # h_nxt (not swapped after l=4) holds the final output
            out_final = h_nxt if NLAYERS in (0, 1, 5) else h_cur
            if NLAYERS == 0:
                out_final = r_sb
            nc.sync.dma_start(
                out=d_out[:].rearrange("(b p) f -> p b f", p=128),
                in_=out_final[:].rearrange("p (b f) -> p b f", f=H),
            )

    lower_extended_insts(nc)

    in_maps = []
    for cc in range(NC):
        in_maps.append(
            {
                "xT": np.ascontiguousarray(xT[cc]),
                "idx16": np.ascontiguousarray(idx16[cc]),
                "drl": np.ascontiguousarray(drl[cc]),
                "w1all": w1all,
                "w2all": w2all,
                "b1c": b1c,
                "garr": garr,
                "barr": barr,
                "b2r": b2r,
                "blkg": blkg,
                "blkb": blkb,
                "tsc": tsc,
                "tbi": tbi,
                "lneps": np.full((128, 1), LN_EPS, np.float32),
                "iota": iota_rep,
                "ident": np.eye(128, dtype=np.float16),
                "encW": encW,
                "encb": encb,
            }
        )
    res = run_bass_kernel_spmd(nc, in_maps, list(range(NC)), trace=_trace)
    LAST_EXEC_NS = res.exec_time_ns
    out = np.empty((N, H), dtype=np.float32)
    for cc in range(NC):
        oc = res.results[cc]["out"]
        out[cc * NPC : (cc + 1) * NPC] = oc[:NPC]
    return out.astype(np.float32)
